# revision 10
# baseline (speedup 1.0000x reference)
"""Trainium2 Bass kernel for nn_MultiHeadAttention_40269613367608.

Sharding: 8 cores = 4 batches x 2 head-groups (tensor-parallel heads,
data-parallel batch). Each core computes its 8 heads of one batch and a
partial output projection; host sums the two partials per batch.

Per-core layouts:
  xT  [C=1024, T]   x transposed (host-prepped, contiguous DMA)
  q,k feature-major [f, t]  (f = head-major (h, half, i)) -> RoPE swap is
      a +-32 partition shift, RMS reduce via ones-matmul on PE
  v   token-major   [t, (h, d)] with a fused ones column per head ->
      AV matmul also produces the softmax denominator l
  PT = exp(scale * K_h^T q_h) computed directly in [tk, tq] orientation,
      so no transposes are ever needed. No max-subtraction: post-RMS rows
      have norm 8, so scaled scores are within [-8, 8] (exp is safe).
"""
import os
import sys

sys.path.insert(0, "/opt/trn_rl_repo")

import numpy as np
import ml_dtypes
from contextlib import ExitStack

import concourse.bass as bass
import concourse.bacc as bacc
import concourse.tile as tile
import concourse.mybir as mybir

F32 = mybir.dt.float32
F32R = mybir.dt.float32r
BF16 = mybir.dt.bfloat16
AF = mybir.ActivationFunctionType

EPS = 1.1920929e-07
C = 1024     # model dim
FH = 512     # per-core features = 8 heads x 64
H = 8        # local heads
D = 64
NCC = C // 128   # contraction chunks for QKV
NFC = FH // 128  # feature chunks (2 heads each)
SCALE = 0.125    # 1/sqrt(D)

# dtype knobs (flip to F32 if accuracy demands)
PT_DT = BF16     # softmax probabilities
V_DT = BF16      # v (AV stationary operand)
YN_DT = BF16     # normalized attention output (out-proj stationary)
WO_DT = BF16     # Wo (must match YN_DT fp32-ness)


def _r(ap):
    """fp32 -> float32r view for full-rate PE matmuls."""
    return ap.bitcast(F32R) if ap.dtype == F32 else ap


def build_nc(T):
    nc = bacc.Bacc(None, target_bir_lowering=False)
    TH = T // 2          # xT streamed in two t-halves
    NTB = TH // 512      # qkv t-blocks per half
    NTT = TH // 128      # v t-tiles per half
    NT = T // 128        # token tiles total
    TQW = 1024           # attention tq block width
    NTQ = T // TQW
    NTK = T // 128       # tk chunks

    xT_d = nc.dram_tensor("xT", [C, T], F32, kind="ExternalInput")
    wqT_d = nc.dram_tensor("wqT", [C, FH], F32, kind="ExternalInput")
    wkT_d = nc.dram_tensor("wkT", [C, FH], F32, kind="ExternalInput")
    wvT_d = nc.dram_tensor("wvT", [C, FH], F32, kind="ExternalInput")
    woT_d = nc.dram_tensor("woT", [FH, C], WO_DT, kind="ExternalInput")
    arep_d = nc.dram_tensor("arep", [128, T], F32, kind="ExternalInput")
    sinT_d = nc.dram_tensor("sinT", [32, T], F32, kind="ExternalInput")
    nsinT_d = nc.dram_tensor("nsinT", [32, T], F32, kind="ExternalInput")
    e2_d = nc.dram_tensor("e2", [128, 2], F32, kind="ExternalInput")
    f2_d = nc.dram_tensor("f2", [2, 128], F32, kind="ExternalInput")
    ones_d = nc.dram_tensor("ones64", [1, 64], F32, kind="ExternalInput")
    out_d = nc.dram_tensor("part", [T, C], F32, kind="ExternalOutput")

    with tile.TileContext(nc) as tc, ExitStack() as ctx:
        const = ctx.enter_context(tc.tile_pool(name="const", bufs=1))
        qkp = ctx.enter_context(tc.tile_pool(name="qk", bufs=1))
        vp = ctx.enter_context(tc.tile_pool(name="vp", bufs=1))

        # constants
        arep = const.tile([128, T], F32)
        sinT = const.tile([32, T], F32)
        nsinT = const.tile([32, T], F32)
        e2 = const.tile([128, 2], F32)
        f2 = const.tile([2, 128], F32)
        ones64 = const.tile([1, 64], F32)
        zb128 = const.tile([128, 1], F32)
        epsb2 = const.tile([2, 1], F32)
        nc.vector.memset(zb128[:], 0.0)
        nc.vector.memset(epsb2[:], EPS)
        nc.sync.dma_start(arep[:], arep_d[:])
        nc.sync.dma_start(sinT[:], sinT_d[:])
        nc.sync.dma_start(nsinT[:], nsinT_d[:])
        nc.sync.dma_start(e2[:], e2_d[:])
        nc.sync.dma_start(f2[:], f2_d[:])
        nc.sync.dma_start(ones64[:], ones_d[:])
        e2r = const.tile([128, 2], F32R)
        f2r = const.tile([2, 128], F32R)
        ones64r = const.tile([1, 64], F32R)
        nc.vector.tensor_copy(e2r[:], e2[:])
        nc.vector.tensor_copy(f2r[:], f2[:])
        nc.vector.tensor_copy(ones64r[:], ones64[:])

        # persistent activations
        qn = [qkp.tile([128, T], F32R, tag=f"qn{fc}", name=f"qn{fc}") for fc in range(NFC)]
        kn = [qkp.tile([128, T], F32R, tag=f"kn{fc}", name=f"kn{fc}") for fc in range(NFC)]
        v_sb = [vp.tile([128, H, 65], V_DT, tag=f"v{t}", name=f"v{t}") for t in range(NT)]
        # ---------------- phase 1: QKV + RoPE + RMSnorm ----------------
        with tc.tile_pool(name="xp", bufs=1) as xp, \
             tc.tile_pool(name="wp", bufs=1) as wp, \
             tc.tile_pool(name="scr", bufs=2) as scr, \
             tc.tile_pool(name="scr2", bufs=2) as scr2, \
             tc.tile_pool(name="qkvps", bufs=3, space="PSUM") as qkv_ps, \
             tc.tile_pool(name="ssps", bufs=2, space="PSUM") as ss_ps, \
             tc.tile_pool(name="bcps", bufs=2, space="PSUM") as bc_ps:

            def rope_rms(ps, dst, col0, W):
                cs = slice(col0, col0 + W)
                t1 = scr.tile([128, 512], F32, tag="t1")
                nc.vector.tensor_mul(t1[:, :W], ps, arep[:, cs])
                r2 = scr.tile([128, 512], F32, tag="r2")
                for hh in range(2):
                    b = 64 * hh
                    nc.vector.tensor_mul(
                        r2[b : b + 32, :W], ps[b + 32 : b + 64, :], sinT[:, cs]
                    )
                    nc.vector.tensor_mul(
                        r2[b + 32 : b + 64, :W], ps[b : b + 32, :], nsinT[:, cs]
                    )
                t3 = scr.tile([128, 512], F32, tag="t3")
                nc.vector.tensor_add(t3[:, :W], t1[:, :W], r2[:, :W])
                sq = scr.tile([128, 512], F32R, tag="sq")
                nc.scalar.activation(sq[:, :W], t3[:, :W], AF.Square, bias=zb128[:])
                ss = ss_ps.tile([2, 512], F32, tag="ss")
                nc.tensor.matmul(
                    ss[:, :W], e2r[:], sq[:, :W], start=True, stop=True
                )
                srt = scr2.tile([2, 512], F32R, tag="srt")
                nc.scalar.activation(
                    srt[:, :W], ss[:, :W], AF.Sqrt, scale=1.0 / D, bias=epsb2[:]
                )
                bc = bc_ps.tile([128, 512], F32, tag="bc")
                nc.tensor.matmul(
                    bc[:, :W], f2r[:], srt[:, :W], start=True, stop=True
                )
                rbc = scr2.tile([128, 512], F32, tag="rbc")
                nc.vector.reciprocal_approx_fast(rbc[:, :W], bc[:, :W])
                nc.vector.tensor_mul(dst[:, cs], t3[:, :W], rbc[:, :W])

            for th in range(2):
                xt = [xp.tile([128, TH], F32R, tag=f"x{cc}", name=f"x{cc}") for cc in range(NCC)]
                for cc in range(NCC):
                    stg = scr.tile([128, TH], F32, tag="stage", name="stage", bufs=3)
                    nc.gpsimd.dma_start(
                        stg[:], xT_d[cc * 128 : (cc + 1) * 128, th * TH : (th + 1) * TH]
                    )
                    nc.gpsimd.tensor_copy(xt[cc][:], stg[:])
                for proj, (w_d, dst) in enumerate(
                    [(wqT_d, qn), (wkT_d, kn), (wvT_d, None)]
                ):
                    wt = [wp.tile([128, FH], F32R, tag=f"w{cc}", name=f"w{cc}") for cc in range(NCC)]
                    for cc in range(NCC):
                        wstg = scr.tile([128, FH], F32, tag="wstage", name="wstage", bufs=3)
                        nc.gpsimd.dma_start(wstg[:], w_d[cc * 128 : (cc + 1) * 128, :])
                        nc.gpsimd.tensor_copy(wt[cc][:], wstg[:])
                    if proj < 2:
                        for fc in range(NFC):
                            for tb in range(NTB):
                                ps = qkv_ps.tile([128, 512], F32, tag="qkvps")
                                for cc in range(NCC):
                                    nc.tensor.matmul(
                                        ps[:],
                                        wt[cc][:, fc * 128 : (fc + 1) * 128],
                                        xt[cc][:, tb * 512 : (tb + 1) * 512],
                                        start=(cc == 0),
                                        stop=(cc == NCC - 1),
                                    )
                                rope_rms(ps[:], dst[fc], th * TH + tb * 512, 512)
                    else:
                        for tt in range(NTT):
                            ps = qkv_ps.tile([128, 512], F32, tag="qkvps")
                            for cc in range(NCC):
                                nc.tensor.matmul(
                                    ps[:],
                                    xt[cc][:, tt * 128 : (tt + 1) * 128],
                                    wt[cc][:],
                                    start=(cc == 0),
                                    stop=(cc == NCC - 1),
                                )
                            gt = th * NTT + tt
                            nc.vector.memset(v_sb[gt][:, :, 64:65], 1.0)
                            nc.vector.tensor_copy(
                                v_sb[gt][:, :, 0:64],
                                ps.rearrange("p (h d) -> p h d", h=H),
                            )

        # ---------------- phases 2+3 SBUF pools ----------------
        ctx2 = ctx.enter_context(ExitStack())
        ptp = ctx2.enter_context(tc.tile_pool(name="ptp", bufs=3))
        ynp = ctx2.enter_context(tc.tile_pool(name="ynp", bufs=1))
        scrA = ctx2.enter_context(tc.tile_pool(name="scrA", bufs=2))
        wop = ctx2.enter_context(tc.tile_pool(name="wop", bufs=1))
        otp = ctx2.enter_context(tc.tile_pool(name="otp", bufs=3))
        yn = [
            [ynp.tile([128, TQW], YN_DT, tag=f"yn{fc}_{tq}", name=f"yn{fc}_{tq}") for fc in range(NFC)]
            for tq in range(NTQ)
        ]

        # ---------------- phase 2: attention ----------------
        with tc.tile_pool(name="stps", bufs=2, space="PSUM") as st_ps, \
             tc.tile_pool(name="ytps", bufs=1, space="PSUM") as yt_ps, \
             tc.tile_pool(name="bcnps", bufs=1, space="PSUM") as bcn_ps:
            for tqb in range(NTQ):
                for h in range(H):
                    fc, hh = h // 2, h % 2
                    rb = 64 * hh
                    yt = yt_ps.tile([65, TQW], F32, tag="yt")
                    for tkc in range(NTK):
                        st = st_ps.tile([128, TQW], F32, tag="st")
                        for hf in range(2):
                            nc.tensor.matmul(
                                st[:, hf * 512 : (hf + 1) * 512],
                                kn[fc][rb : rb + 64, tkc * 128 : (tkc + 1) * 128],
                                qn[fc][
                                    rb : rb + 64,
                                    tqb * TQW + hf * 512 : tqb * TQW + (hf + 1) * 512,
                                ],
                                start=True,
                                stop=True,
                            )
                        pt = ptp.tile([128, TQW], PT_DT, tag="pt")
                        nc.scalar.activation(pt[:], st[:], AF.Exp, scale=SCALE, bias=zb128[:])
                        for hf in range(2):
                            nc.tensor.matmul(
                                yt[:, hf * 512 : (hf + 1) * 512],
                                v_sb[tkc][:, h, :],
                                pt[:, hf * 512 : (hf + 1) * 512],
                                start=(tkc == 0),
                                stop=(tkc == NTK - 1),
                                skip_group_check=True,
                            )
                    lr = scrA.tile([1, TQW], F32R, tag="lr")
                    nc.vector.tensor_copy(lr[:], yt[64:65, :])
                    bcn = bcn_ps.tile([64, TQW], F32, tag="bcn")
                    for hf in range(2):
                        nc.tensor.matmul(
                            bcn[:, hf * 512 : (hf + 1) * 512],
                            ones64r[:],
                            lr[:, hf * 512 : (hf + 1) * 512],
                            start=True,
                            stop=True,
                        )
                    rbcn = scrA.tile([64, TQW], F32, tag="rbcn")
                    nc.vector.reciprocal_approx_fast(rbcn[:], bcn[:])
                    nc.vector.tensor_mul(yn[tqb][fc][rb : rb + 64, :], yt[0:64, :], rbcn[:])

        # ---------------- phase 3: output projection ----------------
        wo = [wop.tile([128, C], WO_DT, tag=f"wo{fc}", name=f"wo{fc}") for fc in range(NFC)]
        for fc in range(NFC):
            nc.sync.dma_start(wo[fc][:], woT_d[fc * 128 : (fc + 1) * 128, :])
        with tc.tile_pool(name="opps", bufs=3, space="PSUM") as op_ps:
            for tqb in range(NTQ):
                for tt in range(TQW // 128):
                    for ob in range(2):
                        op = op_ps.tile([128, 512], F32, tag="op")
                        for fc in range(NFC):
                            nc.tensor.matmul(
                                op[:],
                                yn[tqb][fc][:, tt * 128 : (tt + 1) * 128],
                                wo[fc][:, ob * 512 : (ob + 1) * 512],
                                start=(fc == 0),
                                stop=(fc == NFC - 1),
                            )
                        ot = otp.tile([128, 512], F32, tag="ot")
                        nc.vector.tensor_copy(ot[:], op[:])
                        nc.sync.dma_start(
                            out_d[
                                tqb * TQW + tt * 128 : tqb * TQW + (tt + 1) * 128,
                                ob * 512 : (ob + 1) * 512,
                            ],
                            ot[:],
                        )

    nc.compile()
    return nc


_NC_CACHE = {}


def get_nc(T):
    if T not in _NC_CACHE:
        _NC_CACHE[T] = build_nc(T)
    return _NC_CACHE[T]


def make_in_maps(x, cos, sin, Wq, Wk, Wv, Wo):
    B, T, _ = x.shape
    cosT = np.ascontiguousarray(cos.reshape(T, 32).T)
    sinT = np.ascontiguousarray(sin.reshape(T, 32).T)
    arep = np.tile(cosT, (4, 1))
    nsinT = np.ascontiguousarray(-sinT)
    e2 = np.zeros((128, 2), np.float32)
    e2[0:64, 0] = 1.0
    e2[64:128, 1] = 1.0
    f2 = np.zeros((2, 128), np.float32)
    f2[0, 0:64] = 1.0
    f2[1, 64:128] = 1.0
    ones64 = np.ones((1, 64), np.float32)
    wo_np = lambda a: np.ascontiguousarray(a).astype(ml_dtypes.bfloat16) \
        if WO_DT == BF16 else np.ascontiguousarray(a)
    in_maps = []
    for core in range(8):
        b, g = core // 2, core % 2
        rows = slice(g * FH, (g + 1) * FH)
        in_maps.append(
            {
                "xT": np.ascontiguousarray(x[b].T),
                "wqT": np.ascontiguousarray(Wq[rows].T),
                "wkT": np.ascontiguousarray(Wk[rows].T),
                "wvT": np.ascontiguousarray(Wv[rows].T),
                "woT": wo_np(Wo[:, rows].T),
                "arep": arep,
                "sinT": sinT,
                "nsinT": nsinT,
                "e2": e2,
                "f2": f2,
                "ones64": ones64,
            }
        )
    return in_maps


LAST_RESULT = None


def kernel(x, cos, sin, Wq, Wk, Wv, Wo):
    global LAST_RESULT
    from concourse.bass_utils import run_bass_kernel_spmd

    x = np.asarray(x, dtype=np.float32)
    B, T, Cx = x.shape
    nc = get_nc(T)
    in_maps = make_in_maps(
        x,
        np.asarray(cos, np.float32),
        np.asarray(sin, np.float32),
        np.asarray(Wq, np.float32),
        np.asarray(Wk, np.float32),
        np.asarray(Wv, np.float32),
        np.asarray(Wo, np.float32),
    )
    res = run_bass_kernel_spmd(
        nc, in_maps, list(range(8)), trace=bool(os.environ.get("MHA_TRACE"))
    )
    LAST_RESULT = res
    out = np.empty((B, T, Cx), np.float32)
    for b in range(B):
        out[b] = res.results[2 * b]["part"] + res.results[2 * b + 1]["part"]
    return out
